# revision 1
# baseline (speedup 1.0000x reference)
"""Trainium2 Bass kernel for per-combination linear encoder (embedding lookup).

Computes z = y * w[idx] + b[idx] where idx = t*1024 + x @ [512,256,...,1]
for x in {0,1}^[N,10], t in {0,1}^[N,1], over a 2048-entry (w,b) table.

Sharding: data-parallel over the batch axis across 8 NeuronCores; the
tiny (w,b) table is replicated to every core (and every SBUF partition).

Per-core pipeline (tiles of [128 partitions x B rows], B per tile-schedule):
  1. DMA x/t/y tiles (contiguous per partition, p-major row assignment).
  2. DVE: idx = segmented-reduce(x * powers) + 1024*t, cast to int16.
  3. GPSIMD ap_gather (d=2) against a per-partition interleaved (w,b)
     table: out[p, c*16+q, :] = (w,b)[idx(16k+q, c)] for p in core k
     (each Q7 core gathers its 16 partitions' indices, wrapped order,
     output replicated across the core's partitions).
  4. TensorE un-wrap: 16 accumulating diagonal-mask matmuls per value
     pick og[p, c*16 + p%16] into compact PSUM tiles (exact: masks are
     0/1 so fp32 matmul selection is lossless).
  5. DVE FMA z = y*w + b, DMA out.

The gather dominates (~3.6 ns/row/core of GPSIMD time); all other
engines (DMA ~45us, DVE ~55us, PE ~290us) hide behind it.
"""

import numpy as np

import concourse.bacc as bacc
import concourse.mybir as mybir
from concourse.tile import TileContext
from concourse.bass_utils import run_bass_kernel_spmd

M = 8            # NeuronCores
P = 128          # SBUF partitions
# rows-per-partition schedule: sized to fit og (16*B*2 fp32) double-buffered
# in SBUF; small last tile shortens the post-gather tail (PE+FMA+store).
# RPP=1954 keeps batch padding minimal (N/8 = 250_000 -> 250_112 rows/core).
B_SCHED = (440, 440, 440, 440, 194)
RPP = sum(B_SCHED)          # rows per partition (1954)
R = P * RPP                 # rows per core (250_112)
D = 10           # covariate bits
C = 2048         # table entries
F32 = mybir.dt.float32
I16 = mybir.dt.int16

_CACHE = {}


def _build_program():
    nc = bacc.Bacc("TRN2", target_bir_lowering=False, debug=False, num_devices=M)

    x = nc.dram_tensor("x", [R, D], F32, kind="ExternalInput")
    t = nc.dram_tensor("t", [R], F32, kind="ExternalInput")
    y = nc.dram_tensor("y", [R], F32, kind="ExternalInput")
    wb = nc.dram_tensor("wb", [P, 2 * C], F32, kind="ExternalInput")
    pw = nc.dram_tensor("pw", [P, D], F32, kind="ExternalInput")
    mk = nc.dram_tensor("mk", [P, 16 * P], F32, kind="ExternalInput")
    z = nc.dram_tensor("z", [R], F32, kind="ExternalOutput")

    # row (tile i, partition p, col c) = (off_i*P + p*B_i + c) of the shard
    x3 = x.ap().rearrange("(pp r) d -> pp (r d)", pp=P)   # [P, RPP*D]
    t2 = t.ap().rearrange("(pp r) -> pp r", pp=P)          # [P, RPP]
    y2 = y.ap().rearrange("(pp r) -> pp r", pp=P)
    z2 = z.ap().rearrange("(pp r) -> pp r", pp=P)

    with TileContext(nc) as tc:
        with (
            tc.tile_pool(name="const", bufs=1) as cpool,
            tc.tile_pool(name="sb", bufs=2) as pool,
            tc.tile_pool(name="gat", bufs=2) as gpool,
            tc.tile_pool(name="ps", bufs=2, space="PSUM") as ppool,
        ):
            wb_t = cpool.tile([P, 2 * C], F32)
            nc.sync.dma_start(out=wb_t[:], in_=wb[:, :])
            pw_t = cpool.tile([P, D], F32)
            nc.sync.dma_start(out=pw_t[:], in_=pw[:, :])
            mk_t = cpool.tile([P, 16 * P], F32)
            nc.sync.dma_start(out=mk_t[:], in_=mk[:, :])

            off = 0
            for B in B_SCHED:
                xt = pool.tile([P, B * D], F32, tag="x")
                nc.sync.dma_start(out=xt[:], in_=x3[:, off * D:(off + B) * D])
                tt = pool.tile([P, B], F32, tag="t")
                nc.sync.dma_start(out=tt[:], in_=t2[:, off:off + B])
                yt = pool.tile([P, B], F32, tag="y")
                nc.sync.dma_start(out=yt[:], in_=y2[:, off:off + B])

                # x *= powers (in place; broadcast powers along the row dim)
                xv = xt[:].rearrange("p (b d) -> p b d", d=D)
                nc.vector.tensor_tensor(
                    out=xv, in0=xv,
                    in1=pw_t[:].unsqueeze(1).broadcast_to([P, B, D]),
                    op=mybir.AluOpType.mult,
                )
                # idx = sum_d x*2^(9-d)  (+ 1024*t below)
                idxf = pool.tile([P, B], F32, tag="idxf")
                nc.vector.tensor_reduce(
                    out=idxf[:], in_=xv, axis=mybir.AxisListType.X,
                    op=mybir.AluOpType.add,
                )
                t1024 = pool.tile([P, B], F32, tag="t1024")
                nc.vector.tensor_scalar_mul(out=t1024[:], in0=tt[:], scalar1=1024.0)
                nc.vector.tensor_tensor(
                    out=idxf[:], in0=idxf[:], in1=t1024[:], op=mybir.AluOpType.add
                )
                idx16 = pool.tile([P, B], I16, tag="idx16")
                nc.vector.tensor_copy(out=idx16[:], in_=idxf[:])

                # gather (w,b) pairs: og[p, c*16+q, :] = wb[idx(16k+q, c)]
                og = gpool.tile([P, 16 * B * 2], F32, tag="og")
                nc.gpsimd.ap_gather(
                    out_ap=og[:].rearrange("p (j e) -> p j e", e=2),
                    in_ap=wb_t[:].rearrange("p (c e) -> p c e", e=2),
                    idxs_ap=idx16[:],
                    channels=P, num_elems=C, d=2, num_idxs=16 * B,
                )

                # un-wrap via PE: psum[p, c] = sum_q 1[p%16==q] og[p, (c*16+q)*2+e]
                og3 = og[:].rearrange("p (c s) -> p c s", s=32)
                psw = ppool.tile([P, B], F32, tag="psw")
                psb = ppool.tile([P, B], F32, tag="psb")
                for q in range(16):
                    nc.tensor.matmul(
                        out=psw[:], lhsT=mk_t[:, q * P:(q + 1) * P],
                        rhs=og3[:, :, 2 * q], start=(q == 0), stop=(q == 15),
                    )
                for q in range(16):
                    nc.tensor.matmul(
                        out=psb[:], lhsT=mk_t[:, q * P:(q + 1) * P],
                        rhs=og3[:, :, 2 * q + 1], start=(q == 0), stop=(q == 15),
                    )

                # z = y*w + b
                zt = pool.tile([P, B], F32, tag="z")
                nc.vector.tensor_tensor(
                    out=zt[:], in0=yt[:], in1=psw[:], op=mybir.AluOpType.mult
                )
                nc.vector.tensor_tensor(
                    out=zt[:], in0=zt[:], in1=psb[:], op=mybir.AluOpType.add
                )
                nc.sync.dma_start(out=z2[:, off:off + B], in_=zt[:])
                off += B

    nc.compile()
    return nc


def _get_program():
    if "nc" not in _CACHE:
        _CACHE["nc"] = _build_program()
    return _CACHE["nc"]


def kernel(x, t, y, w, b, trace=False):
    N = x.shape[0]
    npad = M * R - N
    assert npad >= 0
    f32 = np.float32
    # rows assigned per (core, partition, col): shard row index
    # core m gets rows [m*R, (m+1)*R); within a core, partition p holds
    # rows [p*RPP, (p+1)*RPP) of its shard, contiguously.
    xp = np.concatenate([np.asarray(x, f32), np.zeros((npad, D), f32)]).reshape(M, R, D)
    tp = np.concatenate([np.asarray(t, f32).reshape(-1), np.zeros(npad, f32)]).reshape(M, R)
    yp = np.concatenate([np.asarray(y, f32).reshape(-1), np.zeros(npad, f32)]).reshape(M, R)
    wbi = np.empty(2 * C, f32)
    wbi[0::2] = np.asarray(w, f32)
    wbi[1::2] = np.asarray(b, f32)
    wb_rep = np.ascontiguousarray(np.tile(wbi[None, :], (P, 1)))
    pw_rep = np.ascontiguousarray(
        np.tile((2.0 ** np.arange(D - 1, -1, -1)).astype(f32)[None, :], (P, 1))
    )
    mk_host = np.zeros((P, 16 * P), f32)
    for k in range(P):
        mk_host[k, (k % 16) * P + k] = 1.0

    nc = _get_program()
    in_maps = [
        {"x": xp[i], "t": tp[i], "y": yp[i], "wb": wb_rep, "pw": pw_rep, "mk": mk_host}
        for i in range(M)
    ]
    res = run_bass_kernel_spmd(nc, in_maps, core_ids=list(range(M)), trace=trace)
    zfull = np.concatenate([res.results[i]["z"] for i in range(M)])[:N]
    out = zfull.reshape(N, 1).astype(np.float32)
    if trace:
        return out, res
    return out



# revision 2
# speedup vs baseline: 1.8141x; 1.8141x over previous
"""Trainium2 Bass kernel for per-combination linear encoder (embedding lookup).

Computes z = y * w[idx] + b[idx] where idx = t*1024 + x @ [512,256,...,1]
for x in {0,1}^[N,10], t in {0,1}^[N,1], over a 2048-entry (w,b) table.

Sharding: data-parallel over the batch axis across 8 NeuronCores; the
tiny (w,b) table is replicated to every core (and every SBUF partition).

Per-core pipeline (tiles of [128 partitions x B rows]):
  1. DMA xh tiles (bf16, t folded in as an 11th feature column).
  2. DVE: idx = segmented-reduce(xh * powers), cast to int16.
  3. GPSIMD ap_gather (d=2, bf16) against a per-partition interleaved
     (w,b) bf16 table: each Q7 core gathers its 16 partitions' indices
     (wrapped order), output replicated across the core's partitions.
  4. TensorE un-wrap: 16 accumulating diagonal-mask bf16 matmuls per
     value pick og[p, c*16 + p%16] into compact PSUM tiles (masks are
     0/1 so the matmul selection is lossless; w/b are bf16-rounded,
     within the 2e-2 tolerance).
  5. DVE FMA z = y*w + b (fp32), DMA out.

The ap_gather dominates (~21-28 ns per index per Q7 core, serialized
RD_CMD latency); everything else overlaps behind it. bf16 halves the
gather write traffic and makes og small enough to double-buffer, so
the 5 gathers run back-to-back.
"""

import numpy as np
import ml_dtypes

import concourse.bacc as bacc
import concourse.mybir as mybir
from concourse.tile import TileContext
from concourse.bass_utils import run_bass_kernel_spmd

M = 8            # NeuronCores
P = 128          # SBUF partitions
# rows-per-partition schedule: small first tile shortens the pipeline
# lead-in (gather 1 starts after ~15us instead of ~45us).
B_SCHED = (194, 440, 440, 440, 440)
RPP = sum(B_SCHED)          # rows per partition (1954)
R = P * RPP                 # rows per core (250_112)
D = 11           # t + 10 covariate bits, pre-concatenated on host
C = 2048         # table entries
F32 = mybir.dt.float32
BF16 = mybir.dt.bfloat16
I16 = mybir.dt.int16
BF = ml_dtypes.bfloat16

_CACHE = {}


def _build_program():
    nc = bacc.Bacc("TRN2", target_bir_lowering=False, debug=False, num_devices=M)

    x = nc.dram_tensor("x", [R, D], BF16, kind="ExternalInput")
    y = nc.dram_tensor("y", [R], F32, kind="ExternalInput")
    wb = nc.dram_tensor("wb", [P, 2 * C], BF16, kind="ExternalInput")
    pw = nc.dram_tensor("pw", [P, D], BF16, kind="ExternalInput")
    mk = nc.dram_tensor("mk", [P, 16 * P], BF16, kind="ExternalInput")
    z = nc.dram_tensor("z", [R], F32, kind="ExternalOutput")

    # row (tile i, partition p, col c) = (off_i + p*RPP + c) of the shard
    x3 = x.ap().rearrange("(pp r) d -> pp (r d)", pp=P)   # [P, RPP*D]
    y2 = y.ap().rearrange("(pp r) -> pp r", pp=P)
    z2 = z.ap().rearrange("(pp r) -> pp r", pp=P)

    with TileContext(nc) as tc:
        with (
            tc.tile_pool(name="const", bufs=1) as cpool,
            tc.tile_pool(name="in", bufs=3) as ipool,
            tc.tile_pool(name="idx", bufs=3) as xpool,
            tc.tile_pool(name="gat", bufs=2) as gpool,
            tc.tile_pool(name="out", bufs=2) as opool,
            tc.tile_pool(name="ps", bufs=2, space="PSUM") as ppool,
        ):
            wb_t = cpool.tile([P, 2 * C], BF16)
            nc.sync.dma_start(out=wb_t[:], in_=wb[:, :])
            pw_t = cpool.tile([P, D], BF16)
            nc.sync.dma_start(out=pw_t[:], in_=pw[:, :])
            mk_t = cpool.tile([P, 16 * P], BF16)
            nc.sync.dma_start(out=mk_t[:], in_=mk[:, :])

            off = 0
            for B in B_SCHED:
                xt = ipool.tile([P, B * D], BF16, tag="x")
                nc.sync.dma_start(out=xt[:], in_=x3[:, off * D:(off + B) * D])
                yt = ipool.tile([P, B], F32, tag="y")
                nc.sync.dma_start(out=yt[:], in_=y2[:, off:off + B])

                # xh *= powers (in place, bf16 exact: products are 0 or 2^k)
                xv = xt[:].rearrange("p (b d) -> p b d", d=D)
                nc.vector.tensor_tensor(
                    out=xv, in0=xv,
                    in1=pw_t[:].unsqueeze(1).broadcast_to([P, B, D]),
                    op=mybir.AluOpType.mult,
                )
                # idx = sum_d xh*2^(10-d)  (fp32 accumulate, exact)
                idxf = xpool.tile([P, B], F32, tag="idxf")
                nc.vector.tensor_reduce(
                    out=idxf[:], in_=xv, axis=mybir.AxisListType.X,
                    op=mybir.AluOpType.add,
                )
                idx16 = xpool.tile([P, B], I16, tag="idx16")
                nc.vector.tensor_copy(out=idx16[:], in_=idxf[:])

                # gather (w,b) bf16 pairs: og[p, c*16+q, :] = wb[idx(16k+q, c)]
                og = gpool.tile([P, 16 * B * 2], BF16, tag="og")
                nc.gpsimd.ap_gather(
                    out_ap=og[:].rearrange("p (j e) -> p j e", e=2),
                    in_ap=wb_t[:].rearrange("p (c e) -> p c e", e=2),
                    idxs_ap=idx16[:],
                    channels=P, num_elems=C, d=2, num_idxs=16 * B,
                )

                # un-wrap via PE: psum[p, c] = sum_q 1[p%16==q] og[p, (c*16+q)*2+e]
                og3 = og[:].rearrange("p (c s) -> p c s", s=32)
                psw = ppool.tile([P, B], F32, tag="psw")
                psb = ppool.tile([P, B], F32, tag="psb")
                for q in range(16):
                    nc.tensor.matmul(
                        out=psw[:], lhsT=mk_t[:, q * P:(q + 1) * P],
                        rhs=og3[:, :, 2 * q], start=(q == 0), stop=(q == 15),
                    )
                for q in range(16):
                    nc.tensor.matmul(
                        out=psb[:], lhsT=mk_t[:, q * P:(q + 1) * P],
                        rhs=og3[:, :, 2 * q + 1], start=(q == 0), stop=(q == 15),
                    )

                # z = y*w + b
                zt = opool.tile([P, B], F32, tag="z")
                nc.vector.tensor_tensor(
                    out=zt[:], in0=yt[:], in1=psw[:], op=mybir.AluOpType.mult
                )
                nc.vector.tensor_tensor(
                    out=zt[:], in0=zt[:], in1=psb[:], op=mybir.AluOpType.add
                )
                nc.sync.dma_start(out=z2[:, off:off + B], in_=zt[:])
                off += B

    nc.compile()
    return nc


def _get_program():
    if "nc" not in _CACHE:
        _CACHE["nc"] = _build_program()
    return _CACHE["nc"]


def kernel(x, t, y, w, b, trace=False):
    N = x.shape[0]
    npad = M * R - N
    assert npad >= 0
    f32 = np.float32
    # Host-side layout only: fold t in as the MSB feature column, pad,
    # shard rows [m*R, (m+1)*R) to core m; partition p holds rows
    # [p*RPP, (p+1)*RPP) of its shard contiguously.
    xh = np.empty((N + npad, D), BF)
    xh[:N, 0] = np.asarray(t, f32).reshape(-1).astype(BF)
    xh[:N, 1:] = np.asarray(x, f32).astype(BF)
    xh[N:] = 0
    xp = xh.reshape(M, R, D)
    yp = np.concatenate([np.asarray(y, f32).reshape(-1), np.zeros(npad, f32)]).reshape(M, R)
    wbi = np.empty(2 * C, BF)
    wbi[0::2] = np.asarray(w, f32).astype(BF)
    wbi[1::2] = np.asarray(b, f32).astype(BF)
    wb_rep = np.ascontiguousarray(np.tile(wbi[None, :], (P, 1)))
    pw_rep = np.ascontiguousarray(
        np.tile((2.0 ** np.arange(D - 1, -1, -1)).astype(BF)[None, :], (P, 1))
    )
    mk_host = np.zeros((P, 16 * P), BF)
    for k in range(P):
        mk_host[k, (k % 16) * P + k] = 1.0

    nc = _get_program()
    in_maps = [
        {"x": xp[i], "y": yp[i], "wb": wb_rep, "pw": pw_rep, "mk": mk_host}
        for i in range(M)
    ]
    res = run_bass_kernel_spmd(nc, in_maps, core_ids=list(range(M)), trace=trace)
    zfull = np.concatenate([res.results[i]["z"] for i in range(M)])[:N]
    out = zfull.reshape(N, 1).astype(np.float32)
    if trace:
        return out, res
    return out


# revision 3
# speedup vs baseline: 1.8150x; 1.0005x over previous
"""Trainium2 Bass kernel: per-combination linear encoder via PE one-hot
matmuls (no GPSIMD gather).

z = y * w[idx] + b[idx],  idx = t*1024 + x @ [512..1]  (11 bits, 2048 combos)

Split idx = hi5*64 + lo6 (hi5 = t,x0..x3; lo6 = x4..x9). Per tile of
1024 rows (2 groups g of 512 columns, rows on the FREE axis):

  S1a  PE   u_lo[64g+l, c]   = lo6(r) - l          (block-diag affine MM)
  S1b  PE   u_hi[64g+2h+e,c] = hi5(r) - h          (dup e for w/b lanes)
  cp   ACT  u_lo, u_hi -> bf16 SBUF (psum drain, exact: |u|<64)
  S2   DVE  oh = (u_lo == 0)          bf16, 4x mode
  S3   PE   V[64g+2h+e, c] = sum_l T[l,2h+e]*oh    (T = w/b tables, bf16)
  S4   DVE  msk = (u_hi == 0) * V     (fused scalar_tensor_tensor)
  sel  PE   8 tiles accumulate into sel8[32,512]: tile u writes slots
            4u+2g+e via a slot-shifted ones stationary (M=32, N=512)
  FMA  DVE  z[16,512] = y16 * sel8[even] + sel8[odd]; DMA out.

All row/column packing is host-side layout only; the device does all
arithmetic. w/b are bf16 (0.2% rounding, tolerance is 2e-2).
"""

import numpy as np
import ml_dtypes

import concourse.bacc as bacc
import concourse.mybir as mybir
from concourse.tile import TileContext
from concourse.bass_utils import run_bass_kernel_spmd

M = 8              # NeuronCores
NT = 512           # columns per tile (one PSUM bank)
G = 2              # row-groups per column
TPP = 8            # tiles per pack (sel8 accumulation group)
NPACK = 31         # packs per core
NTILES = NPACK * TPP          # 248
RPT = G * NT                  # rows per tile (1024)
R = NTILES * RPT              # rows per core (253952)
C = 2048
F32 = mybir.dt.float32
BF16 = mybir.dt.bfloat16
BF = ml_dtypes.bfloat16

_CACHE = {}


def _build_program():
    nc = bacc.Bacc("TRN2", target_bir_lowering=False, debug=False, num_devices=M)

    xin = nc.dram_tensor("xin", [32, NTILES * NT], BF16, kind="ExternalInput")
    yin = nc.dram_tensor("yin", [NPACK * 16, NT], F32, kind="ExternalInput")
    a1 = nc.dram_tensor("a1", [24, 128], BF16, kind="ExternalInput")
    a2 = nc.dram_tensor("a2", [24, 128], BF16, kind="ExternalInput")
    a3 = nc.dram_tensor("a3", [128, 128], BF16, kind="ExternalInput")
    a4 = nc.dram_tensor("a4", [128, TPP * 48], BF16, kind="ExternalInput")
    z = nc.dram_tensor("z", [NPACK * 16, NT], F32, kind="ExternalOutput")

    isq = mybir.AluOpType.is_equal
    mul = mybir.AluOpType.mult
    add = mybir.AluOpType.add

    with TileContext(nc) as tc:
        with (
            tc.tile_pool(name="const", bufs=1) as cpool,
            tc.tile_pool(name="in", bufs=3) as ipool,
            tc.tile_pool(name="mid", bufs=3) as spool,
            tc.tile_pool(name="msk", bufs=TPP + 2) as mpool,
            tc.tile_pool(name="out", bufs=2) as opool,
            tc.tile_pool(name="ps", bufs=2, space="PSUM") as ppool,
        ):
            a1_t = cpool.tile([24, 128], BF16)
            nc.sync.dma_start(out=a1_t[:], in_=a1[:, :])
            a2_t = cpool.tile([24, 128], BF16)
            nc.sync.dma_start(out=a2_t[:], in_=a2[:, :])
            a3_t = cpool.tile([128, 128], BF16)
            nc.sync.dma_start(out=a3_t[:], in_=a3[:, :])
            a4_t = cpool.tile([128, TPP * 48], BF16)
            nc.sync.dma_start(out=a4_t[:], in_=a4[:, :])

            for pk in range(NPACK):
                xt = ipool.tile([32, TPP * NT], BF16, tag="x")
                nc.sync.dma_start(
                    out=xt[:], in_=xin[:, pk * TPP * NT:(pk + 1) * TPP * NT]
                )
                yt = ipool.tile([16, NT], F32, tag="y")
                nc.sync.dma_start(out=yt[:], in_=yin[16 * pk:16 * (pk + 1), :])

                sel8 = ppool.tile([64, NT], F32, tag="sel")
                msks = []
                for up in range(TPP // 2):
                    u0, u1 = 2 * up, 2 * up + 1
                    xv0 = xt[0:24, u0 * NT:(u0 + 1) * NT]
                    xv1 = xt[0:24, u1 * NT:(u1 + 1) * NT]
                    # ulo in cols [0,NT), uhi in cols [NT,2NT) (adjacent banks);
                    # batch same-stationary matmuls to cut LDWEIGHTS switches
                    uuA = ppool.tile([128, 2 * NT], F32, tag="uu")
                    uuB = ppool.tile([128, 2 * NT], F32, tag="uu")
                    nc.tensor.matmul(
                        out=uuA[:, 0:NT], lhsT=a1_t[:], rhs=xv0, start=True, stop=True
                    )
                    nc.tensor.matmul(
                        out=uuB[:, 0:NT], lhsT=a1_t[:], rhs=xv1, start=True, stop=True
                    )
                    nc.tensor.matmul(
                        out=uuA[:, NT:2 * NT], lhsT=a2_t[:], rhs=xv0,
                        start=True, stop=True
                    )
                    nc.tensor.matmul(
                        out=uuB[:, NT:2 * NT], lhsT=a2_t[:], rhs=xv1,
                        start=True, stop=True
                    )
                    uubA = spool.tile([128, 2 * NT], BF16, tag="uub")
                    nc.scalar.copy(out=uubA[:], in_=uuA[:])
                    uubB = spool.tile([128, 2 * NT], BF16, tag="uub")
                    nc.scalar.copy(out=uubB[:], in_=uuB[:])
                    ohA = spool.tile([128, NT], BF16, tag="oh")
                    nc.vector.tensor_scalar(
                        out=ohA[:], in0=uubA[:, 0:NT], scalar1=0.0,
                        scalar2=None, op0=isq
                    )
                    ohB = spool.tile([128, NT], BF16, tag="oh")
                    nc.vector.tensor_scalar(
                        out=ohB[:], in0=uubB[:, 0:NT], scalar1=0.0,
                        scalar2=None, op0=isq
                    )
                    VA = ppool.tile([128, NT], F32, tag="V")
                    nc.tensor.matmul(
                        out=VA[:], lhsT=a3_t[:], rhs=ohA[:], start=True, stop=True
                    )
                    VB = ppool.tile([128, NT], F32, tag="V")
                    nc.tensor.matmul(
                        out=VB[:], lhsT=a3_t[:], rhs=ohB[:], start=True, stop=True
                    )
                    mskA = mpool.tile([128, NT], BF16, tag="msk")
                    nc.vector.scalar_tensor_tensor(
                        out=mskA[:], in0=uubA[:, NT:2 * NT], scalar=0.0,
                        in1=VA[:], op0=isq, op1=mul,
                    )
                    mskB = mpool.tile([128, NT], BF16, tag="msk")
                    nc.vector.scalar_tensor_tensor(
                        out=mskB[:], in0=uubB[:, NT:2 * NT], scalar=0.0,
                        in1=VB[:], op0=isq, op1=mul,
                    )
                    msks.append(mskA)
                    msks.append(mskB)

                for u in range(TPP):
                    nc.tensor.matmul(
                        out=sel8[0:48, :], lhsT=a4_t[:, 48 * u:48 * (u + 1)],
                        rhs=msks[u][:], start=(u == 0), stop=(u == TPP - 1),
                    )

                # z = y*w + b ; w on sel8 lanes [0:16), b on [32:48)
                zt = opool.tile([16, NT], F32, tag="z")
                nc.vector.tensor_tensor(
                    out=zt[:], in0=yt[:], in1=sel8[0:16, :], op=mul
                )
                nc.vector.tensor_tensor(
                    out=zt[:], in0=zt[:], in1=sel8[32:48, :], op=add
                )
                nc.sync.dma_start(out=z[16 * pk:16 * (pk + 1), :], in_=zt[:])

    nc.compile()
    return nc


def _get_program():
    if "nc" not in _CACHE:
        _CACHE["nc"] = _build_program()
    return _CACHE["nc"]


def _make_consts(w, b):
    f32 = np.float32
    wb_ = np.stack([np.asarray(w, f32), np.asarray(b, f32)], 1).astype(BF)  # [2048, 2]
    a1 = np.zeros((24, 128), BF)
    a2 = np.zeros((24, 128), BF)
    for g in range(G):
        for s in range(6):            # x4..x9 -> lo6, coef 32..1
            a1[12 * g + 5 + s, 64 * g:64 * (g + 1)] = BF(2.0 ** (5 - s))
        a1[12 * g + 11, 64 * g:64 * (g + 1)] = -np.arange(64, dtype=f32).astype(BF)
        for s in range(5):            # t,x0..x3 -> hi5, coef 16..1
            a2[12 * g + s, 64 * g:64 * (g + 1)] = BF(2.0 ** (4 - s))
        hvals = np.repeat(np.arange(32, dtype=f32), 2)
        a2[12 * g + 11, 64 * g:64 * (g + 1)] = (-hvals).astype(BF)
    a3 = np.zeros((128, 128), BF)
    for g in range(G):
        for h in range(32):
            for e in range(2):
                a3[64 * g:64 * g + 64, 64 * g + 2 * h + e] = wb_[h * 64:(h + 1) * 64, e]
    # sel8 slot for tile u, group g: w at partition 2u+g, b at 32+2u+g
    a4 = np.zeros((128, TPP * 48), BF)
    for u in range(TPP):
        for g in range(G):
            for e in range(2):
                for h in range(32):
                    a4[64 * g + 2 * h + e, 48 * u + 32 * e + 2 * u + g] = 1.0
    return a1, a2, a3, a4


def kernel(x, t, y, w, b, trace=False):
    N = x.shape[0]
    Npad = M * R
    npad = Npad - N
    assert npad >= 0
    f32 = np.float32

    # features [12, Npad]: t, x0..x9, ones (bf16; all exact)
    F = np.zeros((12, Npad), BF)
    F[0, :N] = np.asarray(t, f32).reshape(-1).astype(BF)
    F[1:11, :N] = np.asarray(x, f32).T.astype(BF)
    F[11, :N] = BF(1.0)

    xin = np.zeros((M, 32, NTILES * NT), BF)
    yp = np.concatenate([np.asarray(y, f32).reshape(-1), np.zeros(npad, f32)])
    yin = np.empty((M, NPACK * 16, NT), f32)
    for m in range(M):
        Fm = F[:, m * R:(m + 1) * R].reshape(12, NTILES, G, NT)
        xin[m, 0:24] = Fm.transpose(2, 0, 1, 3).reshape(24, NTILES * NT)
        yin[m] = yp[m * R:(m + 1) * R].reshape(NPACK * 16, NT)

    a1, a2, a3, a4 = _make_consts(w, b)

    nc = _get_program()
    in_maps = [
        {"xin": xin[i], "yin": yin[i], "a1": a1, "a2": a2, "a3": a3, "a4": a4}
        for i in range(M)
    ]
    res = run_bass_kernel_spmd(nc, in_maps, core_ids=list(range(M)), trace=trace)
    zfull = np.concatenate(
        [res.results[i]["z"].reshape(-1) for i in range(M)]
    )[:N]
    out = zfull.reshape(N, 1).astype(np.float32)
    if trace:
        return out, res
    return out


# revision 4
# speedup vs baseline: 1.8507x; 1.0197x over previous
"""Trainium2 Bass kernel: per-combination linear encoder via PE one-hot
matmuls (no GPSIMD gather).

z = y * w[idx] + b[idx],  idx = t*1024 + x @ [512..1]  (11 bits, 2048 combos)

Split idx = hi5*64 + lo6 (hi5 = t,x0..x3; lo6 = x4..x9). Per tile of
1024 rows (2 groups g of 512 columns, rows on the FREE axis):

  S1a  PE   u_lo[64g+l, c]   = lo6(r) - l          (block-diag affine MM)
  S1b  PE   u_hi[64g+2h+e,c] = hi5(r) - h          (dup e for w/b lanes)
  cp   ACT  u_lo, u_hi -> bf16 SBUF (psum drain, exact: |u|<64)
  S2   DVE  oh = (u_lo == 0)          bf16, 4x mode
  S3   PE   V[64g+2h+e, c] = sum_l T[l,2h+e]*oh    (T = w/b tables, bf16)
  S4   DVE  msk = (u_hi == 0) * V     (fused scalar_tensor_tensor)
  sel  PE   8 tiles accumulate into sel8[32,512]: tile u writes slots
            4u+2g+e via a slot-shifted ones stationary (M=32, N=512)
  FMA  DVE  z[16,512] = y16 * sel8[even] + sel8[odd]; DMA out.

All row/column packing is host-side layout only; the device does all
arithmetic. w/b are bf16 (0.2% rounding, tolerance is 2e-2).
"""

import numpy as np
import ml_dtypes

import concourse.bacc as bacc
import concourse.mybir as mybir
from concourse.tile import TileContext
from concourse.bass_utils import run_bass_kernel_spmd

M = 8              # NeuronCores
NT = 512           # columns per tile (one PSUM bank)
G = 2              # row-groups per column
TPP = 8            # tiles per pack (sel8 accumulation group)
NPACK = 31         # packs per core
NTILES = NPACK * TPP          # 248
RPT = G * NT                  # rows per tile (1024)
R = NTILES * RPT              # rows per core (253952)
C = 2048
F32 = mybir.dt.float32
BF16 = mybir.dt.bfloat16
BF = ml_dtypes.bfloat16

_CACHE = {}


def _build_program():
    nc = bacc.Bacc("TRN2", target_bir_lowering=False, debug=False, num_devices=M)

    xin = nc.dram_tensor("xin", [32, NTILES * NT], BF16, kind="ExternalInput")
    yin = nc.dram_tensor("yin", [NPACK * 16, NT], F32, kind="ExternalInput")
    a1 = nc.dram_tensor("a1", [24, 128], BF16, kind="ExternalInput")
    a2 = nc.dram_tensor("a2", [24, 128], BF16, kind="ExternalInput")
    a3 = nc.dram_tensor("a3", [128, 128], BF16, kind="ExternalInput")
    a4 = nc.dram_tensor("a4", [128, TPP * 48], BF16, kind="ExternalInput")
    z = nc.dram_tensor("z", [NPACK * 16, NT], F32, kind="ExternalOutput")

    isq = mybir.AluOpType.is_equal
    mul = mybir.AluOpType.mult
    add = mybir.AluOpType.add

    with TileContext(nc) as tc:
        with (
            tc.tile_pool(name="const", bufs=1) as cpool,
            tc.tile_pool(name="in", bufs=4) as ipool,
            tc.tile_pool(name="mid", bufs=6) as spool,
            tc.tile_pool(name="msk", bufs=TPP + 2) as mpool,
            tc.tile_pool(name="out", bufs=2) as opool,
            tc.tile_pool(name="ps", bufs=2, space="PSUM") as ppool,
        ):
            a1_t = cpool.tile([24, 128], BF16)
            nc.sync.dma_start(out=a1_t[:], in_=a1[:, :])
            a2_t = cpool.tile([24, 128], BF16)
            nc.sync.dma_start(out=a2_t[:], in_=a2[:, :])
            a3_t = cpool.tile([128, 128], BF16)
            nc.sync.dma_start(out=a3_t[:], in_=a3[:, :])
            a4_t = cpool.tile([128, TPP * 48], BF16)
            nc.sync.dma_start(out=a4_t[:], in_=a4[:, :])

            for pk in range(NPACK):
                xt = ipool.tile([32, TPP * NT], BF16, tag="x")
                nc.sync.dma_start(
                    out=xt[:], in_=xin[:, pk * TPP * NT:(pk + 1) * TPP * NT]
                )
                yt = ipool.tile([16, NT], F32, tag="y")
                nc.sync.dma_start(out=yt[:], in_=yin[16 * pk:16 * (pk + 1), :])

                sel8 = ppool.tile([64, NT], F32, tag="sel")
                msks = []
                for up in range(TPP // 2):
                    u0, u1 = 2 * up, 2 * up + 1
                    xv0 = xt[0:24, u0 * NT:(u0 + 1) * NT]
                    xv1 = xt[0:24, u1 * NT:(u1 + 1) * NT]
                    # ulo in cols [0,NT), uhi in cols [NT,2NT) (adjacent banks);
                    # batch same-stationary matmuls to cut LDWEIGHTS switches
                    uuA = ppool.tile([128, 2 * NT], F32, tag="uu")
                    uuB = ppool.tile([128, 2 * NT], F32, tag="uu")
                    nc.tensor.matmul(
                        out=uuA[:, 0:NT], lhsT=a1_t[:], rhs=xv0, start=True, stop=True
                    )
                    nc.tensor.matmul(
                        out=uuB[:, 0:NT], lhsT=a1_t[:], rhs=xv1, start=True, stop=True
                    )
                    nc.tensor.matmul(
                        out=uuA[:, NT:2 * NT], lhsT=a2_t[:], rhs=xv0,
                        start=True, stop=True
                    )
                    nc.tensor.matmul(
                        out=uuB[:, NT:2 * NT], lhsT=a2_t[:], rhs=xv1,
                        start=True, stop=True
                    )
                    uubA = spool.tile([128, 2 * NT], BF16, tag="uub")
                    nc.scalar.copy(out=uubA[:], in_=uuA[:])
                    uubB = spool.tile([128, 2 * NT], BF16, tag="uub")
                    nc.scalar.copy(out=uubB[:], in_=uuB[:])
                    ohA = spool.tile([128, NT], BF16, tag="oh")
                    nc.vector.tensor_scalar(
                        out=ohA[:], in0=uubA[:, 0:NT], scalar1=0.0,
                        scalar2=None, op0=isq
                    )
                    ohB = spool.tile([128, NT], BF16, tag="oh")
                    nc.vector.tensor_scalar(
                        out=ohB[:], in0=uubB[:, 0:NT], scalar1=0.0,
                        scalar2=None, op0=isq
                    )
                    VA = ppool.tile([128, NT], F32, tag="V")
                    nc.tensor.matmul(
                        out=VA[:], lhsT=a3_t[:], rhs=ohA[:], start=True, stop=True
                    )
                    VB = ppool.tile([128, NT], F32, tag="V")
                    nc.tensor.matmul(
                        out=VB[:], lhsT=a3_t[:], rhs=ohB[:], start=True, stop=True
                    )
                    mskA = mpool.tile([128, NT], BF16, tag="msk")
                    nc.vector.scalar_tensor_tensor(
                        out=mskA[:], in0=uubA[:, NT:2 * NT], scalar=0.0,
                        in1=VA[:], op0=isq, op1=mul,
                    )
                    mskB = mpool.tile([128, NT], BF16, tag="msk")
                    nc.vector.scalar_tensor_tensor(
                        out=mskB[:], in0=uubB[:, NT:2 * NT], scalar=0.0,
                        in1=VB[:], op0=isq, op1=mul,
                    )
                    msks.append(mskA)
                    msks.append(mskB)

                for u in range(TPP):
                    nc.tensor.matmul(
                        out=sel8[0:48, :], lhsT=a4_t[:, 48 * u:48 * (u + 1)],
                        rhs=msks[u][:], start=(u == 0), stop=(u == TPP - 1),
                    )

                # z = y*w + b ; w on sel8 lanes [0:16), b on [32:48)
                zt = opool.tile([16, NT], F32, tag="z")
                nc.vector.tensor_tensor(
                    out=zt[:], in0=yt[:], in1=sel8[0:16, :], op=mul
                )
                nc.vector.tensor_tensor(
                    out=zt[:], in0=zt[:], in1=sel8[32:48, :], op=add
                )
                nc.sync.dma_start(out=z[16 * pk:16 * (pk + 1), :], in_=zt[:])

    nc.compile()
    return nc


def _get_program():
    if "nc" not in _CACHE:
        _CACHE["nc"] = _build_program()
    return _CACHE["nc"]


def _make_consts(w, b):
    f32 = np.float32
    wb_ = np.stack([np.asarray(w, f32), np.asarray(b, f32)], 1).astype(BF)  # [2048, 2]
    a1 = np.zeros((24, 128), BF)
    a2 = np.zeros((24, 128), BF)
    for g in range(G):
        for s in range(6):            # x4..x9 -> lo6, coef 32..1
            a1[12 * g + 5 + s, 64 * g:64 * (g + 1)] = BF(2.0 ** (5 - s))
        a1[12 * g + 11, 64 * g:64 * (g + 1)] = -np.arange(64, dtype=f32).astype(BF)
        for s in range(5):            # t,x0..x3 -> hi5, coef 16..1
            a2[12 * g + s, 64 * g:64 * (g + 1)] = BF(2.0 ** (4 - s))
        hvals = np.repeat(np.arange(32, dtype=f32), 2)
        a2[12 * g + 11, 64 * g:64 * (g + 1)] = (-hvals).astype(BF)
    a3 = np.zeros((128, 128), BF)
    for g in range(G):
        for h in range(32):
            for e in range(2):
                a3[64 * g:64 * g + 64, 64 * g + 2 * h + e] = wb_[h * 64:(h + 1) * 64, e]
    # sel8 slot for tile u, group g: w at partition 2u+g, b at 32+2u+g
    a4 = np.zeros((128, TPP * 48), BF)
    for u in range(TPP):
        for g in range(G):
            for e in range(2):
                for h in range(32):
                    a4[64 * g + 2 * h + e, 48 * u + 32 * e + 2 * u + g] = 1.0
    return a1, a2, a3, a4


def kernel(x, t, y, w, b, trace=False):
    N = x.shape[0]
    Npad = M * R
    npad = Npad - N
    assert npad >= 0
    f32 = np.float32

    # features [12, Npad]: t, x0..x9, ones (bf16; all exact)
    F = np.zeros((12, Npad), BF)
    F[0, :N] = np.asarray(t, f32).reshape(-1).astype(BF)
    F[1:11, :N] = np.asarray(x, f32).T.astype(BF)
    F[11, :N] = BF(1.0)

    xin = np.zeros((M, 32, NTILES * NT), BF)
    yp = np.concatenate([np.asarray(y, f32).reshape(-1), np.zeros(npad, f32)])
    yin = np.empty((M, NPACK * 16, NT), f32)
    for m in range(M):
        Fm = F[:, m * R:(m + 1) * R].reshape(12, NTILES, G, NT)
        xin[m, 0:24] = Fm.transpose(2, 0, 1, 3).reshape(24, NTILES * NT)
        yin[m] = yp[m * R:(m + 1) * R].reshape(NPACK * 16, NT)

    a1, a2, a3, a4 = _make_consts(w, b)

    nc = _get_program()
    in_maps = [
        {"xin": xin[i], "yin": yin[i], "a1": a1, "a2": a2, "a3": a3, "a4": a4}
        for i in range(M)
    ]
    res = run_bass_kernel_spmd(nc, in_maps, core_ids=list(range(M)), trace=trace)
    zfull = np.concatenate(
        [res.results[i]["z"].reshape(-1) for i in range(M)]
    )[:N]
    out = zfull.reshape(N, 1).astype(np.float32)
    if trace:
        return out, res
    return out
